# revision 3
# baseline (speedup 1.0000x reference)
"""Locally-connected conv (LocalLinear) Trainium2 Bass kernel.

Problem: x (B=64, Cin=64, 32, 32), weight (Cout=64, Cin=64, 32, 32, 3, 3),
bias (Cout=64, 32, 32) -> out (B=64, Cout=64, 32, 32).
out[b,o,y,x] = sum_{c,u,v} xpad[b,c,y+u-1,x+v-1] * W[o,c,y,x,u,v] + bias[o,y,x]

Sharding: spatial rows across 8 cores (core i owns output rows y in
[4i, 4i+4) -> 128 locations/core).  Per location it's an independent
64x64 matmul with contraction 576 = Cin*9.

Per-core kernel layout:
  - x lives on SBUF as xs[128, 6, 34, B] fp16: partitions 0-63 hold
    xpad[c, 4i+r, xi], partitions 64-127 hold the same data shifted one
    column left (xpad[c, 4i+r, xi+1]).  A single moving view
    xs[0:128, r, xi] therefore feeds taps (u,v) and (u,v+1) at once ->
    K=128 matmuls for the tap pairs (u,0)+(u,1), u=0..2.  The three
    v=2 taps stay K=64 (loc A on partitions 0-63; loc B reads the
    shifted top half, which lands on xpad col x_B+2 as needed).
  - weights ship as fp8e3 (e3m4: 4 mantissa bits; measured 1.4e-2 max
    rel err vs the 2e-2 gate) with zero layout padding:
    w[128, 64, 9, 64]; slots 0-2 pair-taps loc A, 3-5 pair-taps loc B,
    6-8 the v=2 singles (loc A bottom / loc B top partitions).
  - locations are paired in PSUM columns (tile_position col 0/64), one
    [128, B] PSUM tile per pair, phases A then B (groups must not
    interleave within a bank).  6 matmuls per phase, 12 per pair.
  - drain: one DVE tensor_scalar_add per pair adds bias (per-partition
    scalar) and casts fp32 PSUM -> fp16 SBUF; output returns fp16 and
    the host casts to fp32.
  - a few dummy warm-up matmuls run during the initial DMA wait so the
    PE p-state is at full clock when real work arrives.
"""

import numpy as np
import ml_dtypes

import concourse.bacc as bacc
import concourse.mybir as mybir
import concourse.tile as tile
from concourse.bass_utils import run_bass_kernel_spmd

NCORES = 8
B = 64
CIN = 64
COUT = 64
H = 32
ROWS_PER_CORE = H // NCORES  # 4
NJ = 64        # loc-pairs per core (4 yy rows x 16 xp)
JB = 8         # loc-pairs per weight DMA block
NBLK = NJ // JB  # 8

F16 = mybir.dt.float16
F32 = mybir.dt.float32
F8 = mybir.dt.float8e3
F8_NP = ml_dtypes.float8_e3m4

_nc_cache = None


def _build_nc():
    from contextlib import ExitStack

    nc = bacc.Bacc("TRN2", target_bir_lowering=False)

    w_d = nc.dram_tensor("w", [128, NJ, 9, 64], F8, kind="ExternalInput")
    xs_d = nc.dram_tensor("xs", [128, 6, 34, B], F16, kind="ExternalInput")
    b_d = nc.dram_tensor("bias_p", [128, NJ], F32, kind="ExternalInput")
    o_d = nc.dram_tensor("out_p", [128, NJ, B], F16, kind="ExternalOutput")

    with tile.TileContext(nc) as tc, ExitStack() as ctx:
        xpool = ctx.enter_context(tc.tile_pool(name="xpool", bufs=1))
        wpool = ctx.enter_context(tc.tile_pool(name="wpool", bufs=3))
        bpool = ctx.enter_context(tc.tile_pool(name="bpool", bufs=1))
        opool = ctx.enter_context(tc.tile_pool(name="opool", bufs=4))
        warmp = ctx.enter_context(tc.tile_pool(name="warmp", bufs=1))
        pspool = ctx.enter_context(tc.tile_pool(name="ps", bufs=6, space="PSUM"))
        wmps = ctx.enter_context(tc.tile_pool(name="wmps", bufs=1, space="PSUM"))

        # PE warm-up: ~3us of dummy matmuls (no DMA deps) so the p-state
        # is fully ramped by the time the first real matmul issues.
        warm = warmp.tile([128, 512], F16)
        nc.vector.memset(warm[:], 0.0)
        wps = wmps.tile([128, 512], F32)
        NWARM = 8
        for k in range(NWARM):
            nc.tensor.matmul(wps[:], warm[:, 0:128], warm[:],
                             start=(k == 0), stop=(k == NWARM - 1))

        bias_sb = bpool.tile([128, NJ], F32)
        nc.sync.dma_start(bias_sb[:], b_d[:])

        xs_sb = xpool.tile([128, 6, 34, B], F16)
        w_sbs = [wpool.tile([128, JB, 9, 64], F8, name=f"w_sb{b_}")
                 for b_ in range(NBLK)]

        # Interleaved input DMA issue order: x chunks land just before
        # the weight blocks that need them; everything streams on the
        # SP (sync) queue in FIFO order.
        def dma_xs_rows(r0, r1, c0=0, c1=34):
            nc.sync.dma_start(xs_sb[:, r0:r1, c0:c1, :],
                              xs_d[:, r0:r1, c0:c1, :])

        def dma_w(blk):
            nc.sync.dma_start(w_sbs[blk][:],
                              w_d[:, blk * JB:(blk + 1) * JB, :, :])

        dma_xs_rows(0, 3, 0, 18)   # blk0 (yy=0, xp 0-7)
        dma_w(0)
        dma_xs_rows(0, 3, 18, 34)  # blk1 (yy=0, xp 8-15)
        dma_w(1)
        dma_w(2)
        dma_xs_rows(3, 4)          # yy=1
        dma_w(3)
        dma_w(4)
        dma_xs_rows(4, 5)          # yy=2
        dma_w(5)
        dma_w(6)
        dma_xs_rows(5, 6)          # yy=3
        dma_w(7)

        for blk in range(NBLK):
            w_sb = w_sbs[blk]
            out_sb = opool.tile([128, JB, B], F16)
            for jj in range(JB):
                j = blk * JB + jj
                yy, xp = divmod(j, 16)
                xA = 2 * xp
                xB = 2 * xp + 1
                ps = pspool.tile([128, B], F32)
                # Phase A: loc xA -> PSUM partitions 0-63.
                for k in range(3):
                    nc.tensor.matmul(
                        ps[0:64, :], w_sb[0:128, jj, k, :],
                        xs_sb[0:128, yy + k, xA, :],
                        start=(k == 0), stop=False,
                        tile_position=(0, 0))
                for k in range(3):
                    nc.tensor.matmul(
                        ps[0:64, :], w_sb[0:64, jj, 6 + k, :],
                        xs_sb[0:64, yy + k, xA + 2, :],
                        start=False, stop=(k == 2),
                        tile_position=(0, 0))
                # Phase B: loc xB -> PSUM partitions 64-127.
                for k in range(3):
                    nc.tensor.matmul(
                        ps[64:128, :], w_sb[0:128, jj, 3 + k, :],
                        xs_sb[0:128, yy + k, xB, :],
                        start=(k == 0), stop=False,
                        tile_position=(0, 64))
                for k in range(3):
                    nc.tensor.matmul(
                        ps[64:128, :], w_sb[64:128, jj, 6 + k, :],
                        xs_sb[64:128, yy + k, xA + 2, :],
                        start=False, stop=(k == 2),
                        tile_position=(64, 64))
                # Drain: +bias (per-partition scalar), fp32 -> fp16.
                nc.vector.tensor_scalar_add(
                    out_sb[:, jj, :], ps[:], bias_sb[:, j:j + 1])
            nc.scalar.dma_start(
                o_d[:, blk * JB:(blk + 1) * JB, :], out_sb[:])

    nc.compile()
    return nc


def get_nc():
    global _nc_cache
    if _nc_cache is None:
        _nc_cache = _build_nc()
    return _nc_cache


def prep_inputs(x, weight, bias):
    """Host-side resharding/relayout -> list of 8 per-core input dicts."""
    x = np.asarray(x, dtype=np.float32)
    weight = np.asarray(weight, dtype=np.float32)
    bias = np.asarray(bias, dtype=np.float32)

    # x slices with halo: xs[i, p, r, xi, b]; p<64: xpad[c, 4i+r, xi],
    # p>=64: xpad[c, 4i+r, xi+1] (column-shifted copy for tap pairing).
    xp_ = np.zeros((B, CIN, H + 2, H + 2), np.float32)
    xp_[:, :, 1:H + 1, 1:H + 1] = x
    xs = np.zeros((NCORES, 128, 6, 34, B), np.float16)
    for i in range(NCORES):
        s = xp_[:, :, 4 * i:4 * i + 6, :].transpose(1, 2, 3, 0)  # (c,6,34,b)
        xs[i, 0:64] = s
        xs[i, 64:128, :, 0:33, :] = s[:, :, 1:34, :]

    # weights: wp[i, p, j=(yy,xp), slot, o]
    #   slot k in {0,1,2}: tap pair (k,0)+(k,1) loc A (bottom v=0, top v=1)
    #   slot 3+k: same for loc B;  slot 6+k: single (k,2), A bottom / B top.
    T = weight.reshape(COUT, CIN, NCORES, 4, 16, 2, 3, 3)
    T = T.transpose(2, 1, 3, 4, 5, 6, 7, 0)  # i c yy xp xe u v o
    bot = np.empty((NCORES, 64, 4, 16, 9, COUT), np.float32)
    top = np.empty((NCORES, 64, 4, 16, 9, COUT), np.float32)
    for k in range(3):
        bot[..., k, :] = T[:, :, :, :, 0, k, 0, :]
        top[..., k, :] = T[:, :, :, :, 0, k, 1, :]
        bot[..., 3 + k, :] = T[:, :, :, :, 1, k, 0, :]
        top[..., 3 + k, :] = T[:, :, :, :, 1, k, 1, :]
        bot[..., 6 + k, :] = T[:, :, :, :, 0, k, 2, :]
        top[..., 6 + k, :] = T[:, :, :, :, 1, k, 2, :]
    wp = np.concatenate([bot, top], axis=1)  # [i, 128, 4, 16, 9, o]
    wp = np.ascontiguousarray(wp.reshape(NCORES, 128, NJ, 9, 64)).astype(F8_NP)

    # bias: bp[i, p, j]; p<64: bias[p, y, xA], p>=64: bias[p-64, y, xB]
    Bb = bias.reshape(COUT, NCORES, 4, 16, 2)  # o i yy xp xe
    bp = np.ascontiguousarray(
        Bb.transpose(1, 4, 0, 2, 3).reshape(NCORES, 128, NJ), dtype=np.float32)

    return [
        {"w": wp[i],
         "xs": np.ascontiguousarray(xs[i]),
         "bias_p": bp[i]}
        for i in range(NCORES)
    ]


def unpack_output(results):
    """results: list of 8 dicts with 'out_p' [128, NJ, B] -> (B, COUT, H, H)."""
    allout = np.stack([np.asarray(r["out_p"], np.float32) for r in results])
    a = allout.reshape(NCORES, 2, COUT, 4, 16, B)     # i xe o yy xp b
    out = a.transpose(5, 2, 0, 3, 4, 1).reshape(B, COUT, H, H)
    return np.ascontiguousarray(out, dtype=np.float32)


def kernel(x, weight, bias, _trace=False, _tmpdir=None):
    nc = get_nc()
    in_maps = prep_inputs(x, weight, bias)
    res = run_bass_kernel_spmd(
        nc, in_maps, core_ids=list(range(NCORES)),
        trace=_trace, tmpdir=_tmpdir,
        **({"trace_cores": list(range(NCORES))} if _trace else {}),
    )
    out = unpack_output(res.results)
    if _trace:
        kernel.last_results = res
    return out


# revision 8
# speedup vs baseline: 1.8876x; 1.8876x over previous
"""Locally-connected conv (LocalLinear) Trainium2 Bass kernel.

Problem: x (B=64, Cin=64, 32, 32), weight (Cout=64, Cin=64, 32, 32, 3, 3),
bias (Cout=64, 32, 32) -> out (B=64, Cout=64, 32, 32).
out[b,o,y,x] = sum_{c,u,v} xpad[b,c,y+u-1,x+v-1] * W[o,c,y,x,u,v] + bias[o,y,x]

Sharding: spatial rows across 8 cores (core i owns output rows y in
[4i, 4i+4) -> 128 locations/core).  Per location it's an independent
64x64 matmul with contraction 576 = Cin*9.

Per-core kernel layout (LoadStationary-bound design):
  The PE power-throttles to ~1.2 GHz when run at high duty, and
  LoadStationary ingests 1 stationary row/cycle -- so total LS rows
  (= weight elements / stationary width M) is the real PE cost.  To get
  M=128 despite Cout=64, each matmul carries TWO adjacent locations'
  weights side-by-side in the stationary columns (cols 0-63 loc A,
  64-127 loc B) over the UNION of their tap windows (contraction =
  (c, window col pair)); weights outside a location's own window are
  zero (33% pad).  6 K=128xM=128 matmuls per location pair, one PSUM
  accumulation group, LS = 6*128 = 768 rows/pair.

  - x lives on SBUF as xs[128, 6, 34, B] fp16: partitions 0-63 hold
    xpad[c, 4i+r, xi], partitions 64-127 the same shifted one column
    left (xpad[c, 4i+r, xi+1]).  Moving view xs[0:128, yy+u, xA+2h]
    supplies window cols (xA+2h, xA+2h+1) for both locations at once.
  - weights ship as fp8e3 (e3m4: 4 mantissa bits; measured 1.4e-2 max
    rel err vs the 2e-2 gate): w[128, 64, 6, 128], slot s = 2u+h.
  - drain: one DVE tensor_scalar_add per pair adds bias (per-partition
    scalar) and casts fp32 PSUM -> fp16 SBUF; output returns fp16 and
    the host casts to fp32.
  - a few dummy warm-up matmuls run during the initial DMA wait so the
    PE p-state is at full clock when real work arrives.
"""

import numpy as np
import ml_dtypes

import concourse.bacc as bacc
import concourse.mybir as mybir
import concourse.tile as tile
from concourse.bass_utils import run_bass_kernel_spmd

NCORES = 8
B = 64
CIN = 64
COUT = 64
H = 32
ROWS_PER_CORE = H // NCORES  # 4
NJ = 64        # loc-pairs per core (4 yy rows x 16 xp)
JB = 8         # loc-pairs per weight DMA block
NBLK = NJ // JB  # 8

F16 = mybir.dt.float16
F32 = mybir.dt.float32
F8 = mybir.dt.float8e3
F8_NP = ml_dtypes.float8_e3m4

_nc_cache = None


def _build_nc():
    from contextlib import ExitStack

    nc = bacc.Bacc("TRN2", target_bir_lowering=False)

    w_d = nc.dram_tensor("w", [128, NJ, 6, 128], F8, kind="ExternalInput")
    xs_d = nc.dram_tensor("xs", [128, 6, 34, B], F16, kind="ExternalInput")
    b_d = nc.dram_tensor("bias_p", [128, NJ], F32, kind="ExternalInput")
    o_d = nc.dram_tensor("out_p", [128, NJ, B], F16, kind="ExternalOutput")

    with tile.TileContext(nc) as tc, ExitStack() as ctx:
        xpool = ctx.enter_context(tc.tile_pool(name="xpool", bufs=1))
        wpool = ctx.enter_context(tc.tile_pool(name="wpool", bufs=3))
        bpool = ctx.enter_context(tc.tile_pool(name="bpool", bufs=1))
        opool = ctx.enter_context(tc.tile_pool(name="opool", bufs=4))
        warmp = ctx.enter_context(tc.tile_pool(name="warmp", bufs=1))
        pspool = ctx.enter_context(tc.tile_pool(name="ps", bufs=6, space="PSUM"))
        wmps = ctx.enter_context(tc.tile_pool(name="wmps", bufs=1, space="PSUM"))

        # PE warm-up: ~3us of dummy matmuls (no DMA deps) so the p-state
        # is fully ramped by the time the first real matmul issues.
        warm = warmp.tile([128, 512], F16)
        nc.vector.memset(warm[:], 0.0)
        wps = wmps.tile([128, 512], F32)
        NWARM = 8
        for k in range(NWARM):
            nc.tensor.matmul(wps[:], warm[:, 0:128], warm[:],
                             start=(k == 0), stop=(k == NWARM - 1))

        bias_sb = bpool.tile([128, NJ], F32)
        nc.sync.dma_start(bias_sb[:], b_d[:])

        xs_sb = xpool.tile([128, 6, 34, B], F16)
        w_sbs = [wpool.tile([128, JB, 6, 128], F8, name=f"w_sb{b_}")
                 for b_ in range(NBLK)]

        # Interleaved input DMA issue order: x chunks land just before
        # the weight blocks that need them; everything streams on the
        # SP (sync) queue in FIFO order.
        def dma_xs_rows(r0, r1, c0=0, c1=34):
            nc.sync.dma_start(xs_sb[:, r0:r1, c0:c1, :],
                              xs_d[:, r0:r1, c0:c1, :])

        def dma_w(blk):
            nc.sync.dma_start(w_sbs[blk][:],
                              w_d[:, blk * JB:(blk + 1) * JB, :, :])

        dma_xs_rows(0, 3, 0, 18)   # blk0 (yy=0, xp 0-7)
        dma_w(0)
        dma_xs_rows(0, 3, 18, 34)  # blk1 (yy=0, xp 8-15)
        dma_w(1)
        dma_w(2)
        dma_xs_rows(3, 4)          # yy=1
        dma_w(3)
        dma_w(4)
        dma_xs_rows(4, 5)          # yy=2
        dma_w(5)
        dma_w(6)
        dma_xs_rows(5, 6)          # yy=3
        dma_w(7)

        for blk in range(NBLK):
            w_sb = w_sbs[blk]
            out_sb = opool.tile([128, JB, B], F16)
            for jj in range(JB):
                j = blk * JB + jj
                yy, xp = divmod(j, 16)
                xA = 2 * xp
                ps = pspool.tile([128, B], F32)
                # 6 union matmuls: slot s = 2u+h covers window cols
                # (xA+2h, xA+2h+1) for taps u of both locations.
                for u in range(3):
                    for h in range(2):
                        s = 2 * u + h
                        nc.tensor.matmul(
                            ps[:, :], w_sb[0:128, jj, s, :],
                            xs_sb[0:128, yy + u, xA + 2 * h, :],
                            start=(s == 0), stop=(s == 5),
                            tile_position=(0, 0))
                # Drain: +bias (per-partition scalar), fp32 -> fp16.
                nc.vector.tensor_scalar_add(
                    out_sb[:, jj, :], ps[:], bias_sb[:, j:j + 1])
            nc.scalar.dma_start(
                o_d[:, blk * JB:(blk + 1) * JB, :], out_sb[:])

    nc.compile()
    return nc


def get_nc():
    global _nc_cache
    if _nc_cache is None:
        _nc_cache = _build_nc()
    return _nc_cache


def prep_inputs(x, weight, bias):
    """Host-side resharding/relayout -> list of 8 per-core input dicts."""
    x = np.asarray(x, dtype=np.float32)
    weight = np.asarray(weight, dtype=np.float32)
    bias = np.asarray(bias, dtype=np.float32)

    # x slices with halo: xs[i, p, r, xi, b]; p<64: xpad[c, 4i+r, xi],
    # p>=64: xpad[c, 4i+r, xi+1] (column-shifted copy for tap pairing).
    xp_ = np.zeros((B, CIN, H + 2, H + 2), np.float32)
    xp_[:, :, 1:H + 1, 1:H + 1] = x
    xs = np.zeros((NCORES, 128, 6, 34, B), np.float16)
    for i in range(NCORES):
        s = xp_[:, :, 4 * i:4 * i + 6, :].transpose(1, 2, 3, 0)  # (c,6,34,b)
        xs[i, 0:64] = s
        xs[i, 64:128, :, 0:33, :] = s[:, :, 1:34, :]

    # weights: wp[i, p, j=(yy,xp), s=2u+h, col]; stationary row p = (half,
    # c), half 0 reads window col xA+2h, half 1 reads xA+2h+1; cols 0-63
    # are loc A's output channels, 64-127 loc B's.  Entries where the
    # window col falls outside a location's own 3-tap window are zero.
    T = weight.reshape(COUT, CIN, NCORES, 4, 16, 2, 3, 3)
    T = T.transpose(2, 1, 3, 4, 5, 6, 7, 0)  # i c yy xp xe u v o
    low = np.zeros((NCORES, 64, 4, 16, 3, 2, 128), np.float32)
    high = np.zeros((NCORES, 64, 4, 16, 3, 2, 128), np.float32)
    for u in range(3):
        low[:, :, :, :, u, 0, 0:64] = T[:, :, :, :, 0, u, 0, :]
        low[:, :, :, :, u, 1, 0:64] = T[:, :, :, :, 0, u, 2, :]
        low[:, :, :, :, u, 1, 64:128] = T[:, :, :, :, 1, u, 1, :]
        high[:, :, :, :, u, 0, 0:64] = T[:, :, :, :, 0, u, 1, :]
        high[:, :, :, :, u, 0, 64:128] = T[:, :, :, :, 1, u, 0, :]
        high[:, :, :, :, u, 1, 64:128] = T[:, :, :, :, 1, u, 2, :]
    wp = np.concatenate([low, high], axis=1)  # [i, 128, 4, 16, 3, 2, 128]
    wp = np.ascontiguousarray(
        wp.reshape(NCORES, 128, NJ, 6, 128)).astype(F8_NP)

    # bias: bp[i, p, j]; p<64: bias[p, y, xA], p>=64: bias[p-64, y, xB]
    Bb = bias.reshape(COUT, NCORES, 4, 16, 2)  # o i yy xp xe
    bp = np.ascontiguousarray(
        Bb.transpose(1, 4, 0, 2, 3).reshape(NCORES, 128, NJ), dtype=np.float32)

    return [
        {"w": wp[i],
         "xs": np.ascontiguousarray(xs[i]),
         "bias_p": bp[i]}
        for i in range(NCORES)
    ]


def unpack_output(results):
    """results: list of 8 dicts with 'out_p' [128, NJ, B] -> (B, COUT, H, H)."""
    allout = np.stack([np.asarray(r["out_p"], np.float32) for r in results])
    a = allout.reshape(NCORES, 2, COUT, 4, 16, B)     # i xe o yy xp b
    out = a.transpose(5, 2, 0, 3, 4, 1).reshape(B, COUT, H, H)
    return np.ascontiguousarray(out, dtype=np.float32)


def kernel(x, weight, bias, _trace=False, _tmpdir=None):
    nc = get_nc()
    in_maps = prep_inputs(x, weight, bias)
    res = run_bass_kernel_spmd(
        nc, in_maps, core_ids=list(range(NCORES)),
        trace=_trace, tmpdir=_tmpdir,
        **({"trace_cores": list(range(NCORES))} if _trace else {}),
    )
    out = unpack_output(res.results)
    if _trace:
        kernel.last_results = res
    return out
